# revision 32
# baseline (speedup 1.0000x reference)
"""Trainium2 Bass kernel for nn_Bone_loss (VarLoss bone-length variance loss).

HW exec ~71.8 us on 8 cores (baseline single-queue version: ~166-200 us).

Strategy (pure data-parallel over 8 cores, 1024 samples each):
  - The only heavy input is `output` [8192,1,64,64] (134 MB). Each sample
    contributes 14 gathered scalars (pred at 14 distinct joints); we use
    gpsimd dma_gather to fetch one 64-element (256 B) chunk per
    (sample, joint): chunk row = ind>>6; the within-chunk offset ind&63 is
    resolved on-chip with an iota-compare mask + multiply + reduce.
  - SWDGE descriptor generation is the critical path (~9 ns/descriptor on a
    Q7 pair). It is spread across all 4 SWDGE queues (one Q7 core pair
    each; pairs run concurrently, instruction retirement is in order). Per
    half (512 samples, int16 row-index limit) the 14 joint slots split
    across two queues in calls of {4,2,1} slots; the descriptor-ring
    carveout is raised to 2560 so a 2048-descriptor call fits. A 16-idx
    dummy gather (indices memset to 0 on-chip) pages the `mlp` Q7 library
    into all 8 cores off the critical path.
  - ind/target/gt_2d/mask are packed on the host into one [S,102] tensor so
    each half loads with ONE wide-descriptor DMA — the scalar/sync HWDGE
    queues are the hidden serial resource; many small strided loads would
    stall mid-kernel consumers for tens of us.
  - The iota-compare masks (independent of gathered data) are precomputed
    into eqm tiles while the gathers run; per gather call only a
    mult + reduce remain on DVE. All pre-gather DVE work is issued before
    the gather instructions.
  - Bones are padded 12 -> 16 (4 uniform groups of 4) so each grouped
    reduction/broadcast is a single instruction; dummy bones have weight 0
    and memset-zeroed inputs so they contribute exactly 0.
  - Per-core result is a [1,8] row of per-lane partials; the host adds
    8x8 values and applies *0.5/B (the scalar "all-reduce").

Layout (per core, S=1024 samples, halves h in {0,1} of 512):
  sample s = 512*h + 128*b + p   (p = partition, b in [0,4), lane l = 4h+b)
  joint slots j in [0,14) -> joints [0,1,2,3,4,5,6,8,11,12,13,14,15,16]
  pack[s, 0:34]=ind(int32 pairs) [34:51]=target [51:85]=gt_2d [85:102]=mask
  g_h[p, j*256 + b*64 + e]; idx_h[p16, j*32 + u] for s' = 16u + p16,
  value s'*64 + (ind>>6), replicated over all eight 16-partition groups
  (queue q's core pair reads partitions [32q, 32q+32)).
  pred/lo cols: q = h*56 + 4j + b;  bone tensors: cols nc(bone)*8 + l with
  nc = [0..9, 12, 13] (bones 10, 11 shifted into group 3's first slots).
"""

import numpy as np

import concourse.bass as bass
import concourse.tile as tile
from concourse import bacc, mybir
from concourse.bass_utils import run_bass_kernel_spmd

NCORES = 8
B = 8192
S = B // NCORES          # samples per core
HS = S // 2              # samples per gather half (int16 row-index limit)
P = 128
PK = 102                 # packed row: 34 ind + 17 tgt + 34 gxy + 17 msk

_JL = [0, 1, 2, 3, 4, 5, 6, 8, 11, 12, 13, 14, 15, 16]      # joints used
_CHUNKS_ALL = [(0, 0, 7), (7, 8, 1), (8, 11, 6)]
# Bones reordered within groups so endpoint position sequences form affine
# runs. Groups stay [0:4], [4:8], [8:10], [10:12].
_ID1 = [2, 3, 4, 5, 11, 12, 15, 16, 1, 4, 14, 11]
_ID2 = [1, 2, 5, 6, 12, 13, 14, 15, 0, 0, 8, 8]
_POS = {j: i for i, j in enumerate(_JL)}
_WB = [1.0, 1.0085885098415446, 1.0, 1.0085885098415446,
       1.0, 1.1375361376887123, 1.0, 1.1375361376887123,
       1.0, 1.0, 1.0, 1.0]
_NCB = [0, 1, 2, 3, 4, 5, 6, 7, 8, 9, 12, 13]   # 16-bone padded columns
_RUNS_E1 = [(0, 4, 2, 1), (4, 2, 8, 1), (6, 2, 12, 1), (8, 1, 1, 1),
            (9, 1, 4, 1), (10, 1, 11, 1), (11, 1, 8, 1)]
_RUNS_E2 = [(0, 2, 1, 1), (2, 2, 5, 1), (4, 4, 9, 1), (8, 2, 0, 0),
            (10, 2, 7, 0)]
_RUNS_D = [(0, 2, 2, 1, 1, 1), (2, 2, 4, 1, 5, 1), (4, 2, 8, 1, 9, 1),
           (6, 2, 12, 1, 11, 1), (8, 1, 1, 1, 0, 1), (9, 1, 4, 1, 0, 1),
           (10, 1, 11, 1, 7, 1), (11, 1, 8, 1, 7, 1)]
_VAR_WEIGHT = 1.0

# gather call plan: (half, queue, slot0, nslots); queue q owns 7 slots of
# one half. Calls are capped at 2 slots (1024 descs) by the SWDGE
# descriptor-ring carveout (fixed by the runtime; larger calls hang in
# await_space).
_CALL_ROUNDS = [
    [(0, 0, 0, 2), (0, 1, 7, 2), (1, 2, 0, 2), (1, 3, 7, 2)],
    [(0, 0, 2, 2), (0, 1, 9, 2), (1, 2, 2, 2), (1, 3, 9, 2)],
    [(0, 0, 4, 2), (0, 1, 11, 2), (1, 2, 4, 2), (1, 3, 11, 2)],
    [(0, 0, 6, 1), (0, 1, 13, 1), (1, 2, 6, 1), (1, 3, 13, 1)],
]

_F32 = mybir.dt.float32
_I32 = mybir.dt.int32
_I16 = mybir.dt.int16


def _ap(base_ap, dims, off=0):
    """Custom AP: keep base partition dim, override free dims; offset in elems."""
    return bass.AP(base_ap.tensor, base_ap.offset + off,
                   [list(base_ap.ap[0])] + [list(d) for d in dims])


def _dap(base_ap, dims, off=0):
    """Custom DRAM AP with ALL dims explicit (first dim included)."""
    return bass.AP(base_ap.tensor, base_ap.offset + off,
                   [list(d) for d in dims])


def _consts():
    u = np.arange(32, dtype=np.int32)
    p16 = np.arange(16, dtype=np.int32)
    # (16u + p) * 4096: (ind + s'*4096) >> 6 == (ind>>6) + s'*64
    c_base2 = ((16 * u[None, :] + p16[:, None]) * 4096).astype(np.int32)
    c_all = np.zeros((P, 81), np.float32)
    c_all[:, 0:64] = np.arange(64, dtype=np.float32)[None, :]
    w2 = np.zeros(16, np.float32)
    for b, c in enumerate(_NCB):
        w2[c] = np.float32(_WB[b]) ** 2
    c_all[:, 64:80] = w2[None, :]
    c_all[:, 80] = 1.0
    return {"c_base2": c_base2, "c_all": c_all}


def _build_nc():
    nc = bacc.Bacc("TRN2", target_bir_lowering=False, debug=False,
                   enable_asserts=False, num_devices=NCORES,
                   num_swdge_queues=4)
    outv = nc.dram_tensor("outv", [S * 64, 64], _F32, kind="ExternalInput").ap()
    packv = nc.dram_tensor("packv", [S, PK], _I32, kind="ExternalInput").ap()
    c_base2 = nc.dram_tensor("c_base2", [16, 32], _I32,
                             kind="ExternalInput").ap()
    c_all = nc.dram_tensor("c_all", [P, 81], _F32, kind="ExternalInput").ap()
    res = nc.dram_tensor("res", [1, 8], _F32, kind="ExternalOutput").ap()

    AL = mybir.AluOpType
    X = mybir.AxisListType.X
    with tile.TileContext(nc) as tc:
        with tc.tile_pool(name="sbuf", bufs=1) as pool, \
             tc.tile_pool(name="psum", bufs=1, space="PSUM") as psum_pool:
            from concourse import library_config
            nc.gpsimd.load_library(library_config.mlp)

            base2_t = pool.tile([16, 32], _I32)
            call_t = pool.tile([P, 81], _F32)
            iota_v = call_t[:, 0:64]
            zidx_t = pool.tile([32, 8], _I16)
            dump_t = pool.tile([P, 64], _F32)

            idx0 = pool.tile([128, 448], _I16, tag="idx0")
            idx1 = pool.tile([128, 448], _I16, tag="idx1")
            idx_tiles = {0: idx0, 1: idx1}
            g0 = pool.tile([P, 3584], _F32, tag="g0")
            g1 = pool.tile([P, 3584], _F32, tag="g1")
            g_tiles = {0: g0, 1: g1}
            eqm0 = pool.tile([P, 3584], _F32, tag="eqm0")
            eqm1 = pool.tile([P, 3584], _F32, tag="eqm1")
            eqm_tiles = {0: eqm0, 1: eqm1}
            b_pred = pool.tile([P, 112], _F32)
            lof = pool.tile([P, 112], _F32)
            b_all = pool.tile([P, 816], _F32)   # [h*408 + b*102 + c]
            t1raw = pool.tile([16, 2176], _I32)
            bg_tv = pool.tile([P, 768], _F32)
            pd = pool.tile([P, 128], _F32)

            # ----- phase 0.05: library page-in dummy -------------------------
            with tc.tile_wait_until(0.05):
                nc.vector.memset(zidx_t[:], 0)
                # 16-descriptor dummy gather: every Q7 core decodes it and
                # pages the mlp library text in, off the critical path
                nc.gpsimd.dma_gather(
                    _ap(dump_t[:], [[64, 1], [1, 64]]),
                    outv[0:HS * 64, :],
                    zidx_t[0:32, 0:1],
                    16, 16, 64, elem_step=64,
                    queue_num=0,
                )

            # ----- phase 0.1: input DMAs (one per queue-slot where possible)
            with tc.tile_wait_until(0.1):
                # half 0 of the sample-wrapped ind load is split across both
                # HWDGE queues: its sem gates the whole idx chain
                nc.sync.dma_start(
                    _ap(t1raw[:], [[34, 16], [1, 34]], off=0),
                    _dap(packv[:], [[PK, 16], [16 * PK, 16], [1, 34]], off=0))
                nc.scalar.dma_start(
                    _ap(t1raw[:], [[34, 16], [1, 34]], off=544),
                    _dap(packv[:], [[PK, 16], [16 * PK, 16], [1, 34]],
                         off=16 * 16 * PK))
                nc.scalar.dma_start(
                    _ap(t1raw[:], [[34, 32], [1, 34]], off=1088),
                    _dap(packv[:], [[PK, 16], [16 * PK, 32], [1, 34]],
                         off=512 * PK))
                nc.sync.dma_start(base2_t[:], c_base2[:])
                nc.sync.dma_start(call_t[:], c_all[:])
                # pack loads run while the scalar queue would otherwise idle
                # waiting on the idx casts that gate the h1 replicas
                for h in range(2):
                    nc.scalar.dma_start(
                        _ap(b_all[:], [[102, 4], [1, PK]], off=h * 408),
                        _dap(packv[:], [[PK, 128], [128 * PK, 4], [1, PK]],
                             off=(512 * h) * PK).bitcast(_F32))

            # ----- phase 0.15: dummy-bone zeroing ---------------------------
            with tc.tile_wait_until(0.15):
                nc.vector.memset(bg_tv[:], 0.0)
                nc.vector.memset(pd[:], 0.0)

            # ----- phase 0.2: idx math + casts (h0 first), replication ------
            with tc.tile_wait_until(0.2):
                for h in range(2):
                    ev = _ap(t1raw[:], [[34, 32], [2, 17]], off=h * 1088)
                    nc.vector.tensor_tensor(
                        out=ev, in0=ev,
                        in1=_ap(base2_t[:], [[1, 32], [0, 17]]), op=AL.add)
                    nc.vector.tensor_scalar(out=ev, in0=ev, scalar1=6,
                                            scalar2=None,
                                            op0=AL.logical_shift_right)
                    it = idx_tiles[h]
                    for ci, (jt, j0, cnt) in enumerate(_CHUNKS_ALL):
                        nc.vector.tensor_copy(
                            out=_ap(it[0:16, :], [[32, cnt], [1, 32]],
                                    off=jt * 32),
                            in_=_ap(t1raw[:], [[2, cnt], [34, 32]],
                                    off=h * 1088 + 2 * j0))
                        if h == 0 and ci == 0:
                            # partial [16:32] replica right after the first
                            # cast: it alone gates round-1 q0 (cols 0:64),
                            # pulling the whole gather pipeline earlier
                            nc.sync.dma_start(it[16:32, 0:224],
                                              it[0:16, 0:224])
                # replicate [0:16] to every other 16-partition group with
                # independent flat DMAs (no chained sem latency)
                nc.sync.dma_start(idx0[16:32, 224:448], idx0[0:16, 224:448])
                for p0 in range(32, 128, 16):
                    nc.sync.dma_start(idx0[p0:p0 + 16, 0:448],
                                      idx0[0:16, 0:448])
                for p0 in range(16, 128, 16):
                    nc.scalar.dma_start(idx1[p0:p0 + 16, 0:448],
                                        idx1[0:16, 0:448])

            # ----- phase 0.3: lo = ind & 63 ---------------------------------
            with tc.tile_wait_until(0.3):
                for h in range(2):
                    iv = _ap(b_all[:], [[102, 4], [2, 17]],
                             off=h * 408).bitcast(_I32)
                    nc.vector.tensor_scalar(out=iv, in0=iv, scalar1=63,
                                            scalar2=None, op0=AL.bitwise_and)
                    for (jt, j0, cnt) in _CHUNKS_ALL:
                        nc.vector.tensor_copy(
                            out=_ap(lof[:], [[4, cnt], [1, 4]],
                                    off=h * 56 + jt * 4),
                            in_=_ap(b_all[:], [[2, cnt], [102, 4]],
                                    off=h * 408 + 2 * j0).bitcast(_I32))

            # ----- phase 0.5: eq-mask precompute (hidden under gathers) -----
            with tc.tile_wait_until(0.5):
                for h in range(2):
                    nc.vector.tensor_tensor(
                        out=eqm_tiles[h][:].rearrange("p (a e) -> p a e", e=64),
                        in0=_ap(iota_v, [[0, 56], [1, 64]]),
                        in1=_ap(lof[:], [[1, 56], [0, 64]], off=h * 56),
                        op=AL.is_equal)

            # ----- phase 0.6: early bone math (target/gt_2d only) -----------
            xy2 = pool.tile([P, 128], _F32)
            vw2 = pool.tile([P, 128], _F32)
            rn = pool.tile([P, 32], _F32)
            gv = pool.tile([P, 128], _F32)
            with tc.tile_wait_until(0.6):
                for e, runs in enumerate((_RUNS_E1, _RUNS_E2)):
                    for (b0, ln, q0, st) in runs:
                        j0 = _JL[q0]
                        nc.vector.tensor_copy(
                            out=_ap(bg_tv[:], [[8, ln], [4, 2], [1, 4]],
                                    off=e * 128 + _NCB[b0] * 8),
                            in_=_ap(b_all[:], [[st, ln], [408, 2], [102, 4]],
                                    off=34 + j0))
                        nc.vector.tensor_copy(
                            out=_ap(bg_tv[:], [[16, ln], [8, 2], [4, 2], [1, 4]],
                                    off=256 + e * 256 + _NCB[b0] * 16),
                            in_=_ap(b_all[:], [[2 * st, ln], [1, 2], [408, 2],
                                               [102, 4]],
                                    off=51 + 2 * j0))
                n128 = [[1, 128]]
                vis = pool.tile([P, 128], _F32)
                v2 = pool.tile([P, 128], _F32)
                nc.vector.tensor_scalar(out=vis[:], in0=_ap(bg_tv[:], n128, off=0),
                                        scalar1=0.5, scalar2=None, op0=AL.is_gt)
                nc.vector.tensor_scalar(out=v2[:],
                                        in0=_ap(bg_tv[:], n128, off=128),
                                        scalar1=0.5, scalar2=None, op0=AL.is_gt)
                nc.vector.tensor_tensor(out=vis[:], in0=vis[:], in1=v2[:],
                                        op=AL.mult)
                # vw2 = vis * w^2  (fold bone weight into d2 before the sqrt)
                nc.vector.tensor_tensor(
                    out=vw2[:].rearrange("p (a b) -> p a b", a=16),
                    in0=vis[:].rearrange("p (a b) -> p a b", a=16),
                    in1=_ap(call_t[:], [[1, 16], [0, 8]], off=64), op=AL.mult)
                dx = pool.tile([P, 128], _F32)
                dy = pool.tile([P, 128], _F32)
                nc.vector.tensor_tensor(
                    out=dx[:].rearrange("p (a b) -> p a b", a=16),
                    in0=_ap(bg_tv[:], [[16, 16], [1, 8]], off=256),
                    in1=_ap(bg_tv[:], [[16, 16], [1, 8]], off=512),
                    op=AL.subtract)
                nc.vector.tensor_tensor(
                    out=dy[:].rearrange("p (a b) -> p a b", a=16),
                    in0=_ap(bg_tv[:], [[16, 16], [1, 8]], off=264),
                    in1=_ap(bg_tv[:], [[16, 16], [1, 8]], off=520),
                    op=AL.subtract)
                nc.vector.tensor_tensor(out=dx[:], in0=dx[:], in1=dx[:],
                                        op=AL.mult)
                nc.vector.tensor_tensor(out=dy[:], in0=dy[:], in1=dy[:],
                                        op=AL.mult)
                nc.vector.tensor_tensor(out=xy2[:], in0=dx[:], in1=dy[:],
                                        op=AL.add)
                # num = bones visible per group; rn = 1/max(num, 1)
                num = pool.tile([P, 32], _F32)
                nc.vector.tensor_reduce(
                    out=_ap(num[:], [[8, 4], [1, 8]]),
                    in_=_ap(vis[:], [[32, 4], [1, 8], [8, 4]]),
                    axis=X, op=AL.add)
                nc.vector.tensor_scalar(out=num[:], in0=num[:], scalar1=1.0,
                                        scalar2=None, op0=AL.max)
                nc.vector.reciprocal(out=rn[:], in_=num[:])
                # active-sample mask from packed mask columns
                msum = pool.tile([P, 8], _F32)
                nc.vector.tensor_reduce(
                    out=msum[:],
                    in_=_ap(b_all[:], [[102, 8], [1, 17]], off=85),
                    axis=X, op=AL.add)
                nc.vector.tensor_scalar(out=msum[:], in0=msum[:], scalar1=0.0,
                                        scalar2=None, op0=AL.is_equal)
                rn_m = pool.tile([P, 32], _F32)
                nc.vector.tensor_tensor(out=rn_m[:], in0=rn[:],
                                        in1=_ap(msum[:], [[0, 4], [1, 8]]),
                                        op=AL.mult)
                nc.vector.tensor_tensor(
                    out=_ap(gv[:], [[32, 4], [8, 4], [1, 8]]),
                    in0=_ap(vis[:], [[32, 4], [8, 4], [1, 8]]),
                    in1=_ap(rn_m[:], [[8, 4], [0, 4], [1, 8]]), op=AL.mult)

            # ----- gathers: 3 rounds x 4 queues -----------------------------
            def emit_gather(h, q, s0, ns, ph):
                with tc.tile_wait_until(ph):
                    nc.gpsimd.dma_gather(
                        _ap(g_tiles[h][:], [[64, ns * 4], [1, 64]],
                            off=s0 * 256),
                        outv[h * HS * 64:(h + 1) * HS * 64, :],
                        idx_tiles[h][0:32 * (q + 1), s0 * 32:(s0 + ns) * 32],
                        ns * 512, ns * 512, 64, elem_step=64,
                        queue_num=q,
                    )

            def emit_select(h, s0, ns, ph):
                # one mult + one reduce cover BOTH queues of a half: queue
                # q+1's slots sit exactly 7 slots (1792 cols) after queue q's
                with tc.tile_wait_until(ph):
                    eview = _ap(eqm_tiles[h][:], [[1792, 2], [1, ns * 256]],
                                off=s0 * 256)
                    nc.vector.tensor_tensor(
                        out=eview, in0=eview,
                        in1=_ap(g_tiles[h][:], [[1792, 2], [1, ns * 256]],
                                off=s0 * 256),
                        op=AL.mult)
                    nc.vector.tensor_reduce(
                        out=_ap(b_pred[:], [[28, 2], [1, ns * 4]],
                                off=h * 56 + s0 * 4),
                        in_=_ap(eqm_tiles[h][:], [[1792, 2], [64, ns * 4],
                                                  [1, 64]],
                                off=s0 * 256),
                        axis=X, op=AL.add)

            for ri, calls in enumerate(_CALL_ROUNDS):
                for ci, (h, q, s0, ns) in enumerate(calls):
                    emit_gather(h, q, s0, ns, 1.0 + ri + 0.01 * ci)
            for ri, calls in enumerate(_CALL_ROUNDS):
                for hi, (h, q, s0, ns) in enumerate(calls[::2]):
                    emit_select(h, s0, ns, 1.5 + ri + 0.01 * hi)

            # ----- late bone math (needs pred) ------------------------------
            with tc.tile_wait_until(6.0):
                for (b0, ln, p1, s1, p2, s2) in _RUNS_D:
                    nc.vector.tensor_tensor(
                        out=_ap(pd[:], [[8, ln], [4, 2], [1, 4]],
                                off=_NCB[b0] * 8),
                        in0=_ap(b_pred[:], [[4 * s1, ln], [56, 2], [1, 4]],
                                off=p1 * 4),
                        in1=_ap(b_pred[:], [[4 * s2, ln], [56, 2], [1, 4]],
                                off=p2 * 4),
                        op=AL.subtract)
                d2 = pool.tile([P, 128], _F32)
                nc.vector.tensor_tensor(out=d2[:], in0=pd[:], in1=pd[:],
                                        op=AL.mult)
                nc.vector.tensor_tensor(out=d2[:], in0=d2[:], in1=xy2[:],
                                        op=AL.add)
                nc.vector.tensor_tensor(out=d2[:], in0=d2[:], in1=vw2[:],
                                        op=AL.mult)
                ell = pool.tile([P, 128], _F32)
                nc.scalar.sqrt(out=ell[:], in_=d2[:])
                # gate = (d2 > 0) * gv, on DVE while ACT does the sqrt
                gt = pool.tile([P, 128], _F32)
                nc.vector.tensor_scalar(out=gt[:], in0=d2[:], scalar1=0.0,
                                        scalar2=None, op0=AL.is_gt)
                nc.vector.tensor_tensor(out=gt[:], in0=gt[:], in1=gv[:],
                                        op=AL.mult)
                sum_l = pool.tile([P, 32], _F32)
                nc.vector.tensor_reduce(
                    out=_ap(sum_l[:], [[8, 4], [1, 8]]),
                    in_=_ap(ell[:], [[32, 4], [1, 8], [8, 4]]),
                    axis=X, op=AL.add)
                e_t = pool.tile([P, 32], _F32)
                nc.vector.tensor_tensor(out=e_t[:], in0=sum_l[:], in1=rn[:],
                                        op=AL.mult)
                eb = pool.tile([P, 128], _F32)
                nc.vector.tensor_tensor(
                    out=_ap(eb[:], [[32, 4], [8, 4], [1, 8]]),
                    in0=_ap(ell[:], [[32, 4], [8, 4], [1, 8]]),
                    in1=_ap(e_t[:], [[8, 4], [0, 4], [1, 8]]), op=AL.subtract)
                nc.vector.tensor_tensor(out=eb[:], in0=eb[:], in1=eb[:],
                                        op=AL.mult)
                nc.vector.tensor_tensor(out=eb[:], in0=eb[:], in1=gt[:],
                                        op=AL.mult)
                pl = pool.tile([P, 8], _F32)
                nc.vector.tensor_reduce(out=pl[:],
                                        in_=_ap(eb[:], [[1, 8], [8, 16]]),
                                        axis=X, op=AL.add)
                ps = psum_pool.tile([1, 8], _F32, space="PSUM")
                nc.tensor.matmul(out=ps[:], lhsT=call_t[:, 80:81], rhs=pl[:],
                                 start=True, stop=True)
                tot = pool.tile([1, 8], _F32)
                nc.vector.tensor_copy(out=tot[:], in_=ps[:])
                nc.sync.dma_start(res[:], tot[0:1, :])
    nc.compile()
    return nc


_NC_CACHE = None
LAST_RESULTS = None


def kernel(output, mask, ind, target, gt_2d):
    global _NC_CACHE, LAST_RESULTS
    if _NC_CACHE is None:
        _NC_CACHE = _build_nc()
    nc = _NC_CACHE

    output = np.ascontiguousarray(np.asarray(output), dtype=np.float32)
    mask = np.ascontiguousarray(np.asarray(mask), dtype=np.float32)
    target = np.ascontiguousarray(np.asarray(target), dtype=np.float32)
    gt_2d = np.ascontiguousarray(
        np.asarray(gt_2d), dtype=np.float32).reshape(B, 34)
    ind = np.ascontiguousarray(np.asarray(ind))
    if ind.dtype != np.int64:
        ind = ind.astype(np.int64)

    pack = np.empty((B, PK), np.int32)
    pack[:, 0:34] = ind.view(np.int32).reshape(B, 34)
    pack[:, 34:51] = target.view(np.int32)
    pack[:, 51:85] = np.ascontiguousarray(gt_2d).view(np.int32)
    pack[:, 85:102] = mask.view(np.int32)

    consts = _consts()
    in_maps = []
    for c in range(NCORES):
        sl = slice(c * S, (c + 1) * S)
        in_maps.append({
            "outv": np.ascontiguousarray(output[sl]).reshape(S * 64, 64),
            "packv": np.ascontiguousarray(pack[sl]),
            **consts,
        })
    res = run_bass_kernel_spmd(nc, in_maps, core_ids=list(range(NCORES)))
    LAST_RESULTS = res
    total = sum(float(res.results[c]["res"].sum()) for c in range(NCORES))
    return np.asarray([_VAR_WEIGHT * total * 0.5 / B], dtype=np.float32)


# revision 33
# speedup vs baseline: 1.0395x; 1.0395x over previous
"""Trainium2 Bass kernel for nn_Bone_loss (VarLoss bone-length variance loss).

HW exec ~71.8 us on 8 cores (baseline single-queue version: ~166-200 us).

Strategy (pure data-parallel over 8 cores, 1024 samples each):
  - The only heavy input is `output` [8192,1,64,64] (134 MB). Each sample
    contributes 14 gathered scalars (pred at 14 distinct joints); we use
    gpsimd dma_gather to fetch one 64-element (256 B) chunk per
    (sample, joint): chunk row = ind>>6; the within-chunk offset ind&63 is
    resolved on-chip with an iota-compare mask + multiply + reduce.
  - SWDGE descriptor generation is the critical path (~9 ns/descriptor on a
    Q7 pair). It is spread across all 4 SWDGE queues (one Q7 core pair
    each; pairs run concurrently, instruction retirement is in order). Per
    half (512 samples, int16 row-index limit) the 14 joint slots split
    across two queues in calls of {4,2,1} slots; the descriptor-ring
    carveout is raised to 2560 so a 2048-descriptor call fits. A 16-idx
    dummy gather (indices memset to 0 on-chip) pages the `mlp` Q7 library
    into all 8 cores off the critical path.
  - ind/target/gt_2d/mask are packed on the host into one [S,102] tensor so
    each half loads with ONE wide-descriptor DMA — the scalar/sync HWDGE
    queues are the hidden serial resource; many small strided loads would
    stall mid-kernel consumers for tens of us.
  - The iota-compare masks (independent of gathered data) are precomputed
    into eqm tiles while the gathers run; per gather call only a
    mult + reduce remain on DVE. All pre-gather DVE work is issued before
    the gather instructions.
  - Bones are padded 12 -> 16 (4 uniform groups of 4) so each grouped
    reduction/broadcast is a single instruction; dummy bones have weight 0
    and memset-zeroed inputs so they contribute exactly 0.
  - Per-core result is a [1,8] row of per-lane partials; the host adds
    8x8 values and applies *0.5/B (the scalar "all-reduce").

Layout (per core, S=1024 samples, halves h in {0,1} of 512):
  sample s = 512*h + 128*b + p   (p = partition, b in [0,4), lane l = 4h+b)
  joint slots j in [0,14) -> joints [0,1,2,3,4,5,6,8,11,12,13,14,15,16]
  pack[s, 0:34]=ind(int32 pairs) [34:51]=target [51:85]=gt_2d [85:102]=mask
  g_h[p, j*256 + b*64 + e]; idx_h[p16, j*32 + u] for s' = 16u + p16,
  value s'*64 + (ind>>6), replicated over all eight 16-partition groups
  (queue q's core pair reads partitions [32q, 32q+32)).
  pred/lo cols: q = h*56 + 4j + b;  bone tensors: cols nc(bone)*8 + l with
  nc = [0..9, 12, 13] (bones 10, 11 shifted into group 3's first slots).
"""

import numpy as np

import concourse.bass as bass
import concourse.tile as tile
from concourse import bacc, mybir
from concourse.bass_utils import run_bass_kernel_spmd

NCORES = 8
B = 8192
S = B // NCORES          # samples per core
HS = S // 2              # samples per gather half (int16 row-index limit)
P = 128
PK = 102                 # packed row: 34 ind + 17 tgt + 34 gxy + 17 msk

_JL = [0, 1, 2, 3, 4, 5, 6, 8, 11, 12, 13, 14, 15, 16]      # joints used
_CHUNKS_ALL = [(0, 0, 7), (7, 8, 1), (8, 11, 6)]
# Bones reordered within groups so endpoint position sequences form affine
# runs. Groups stay [0:4], [4:8], [8:10], [10:12].
_ID1 = [2, 3, 4, 5, 11, 12, 15, 16, 1, 4, 14, 11]
_ID2 = [1, 2, 5, 6, 12, 13, 14, 15, 0, 0, 8, 8]
_POS = {j: i for i, j in enumerate(_JL)}
_WB = [1.0, 1.0085885098415446, 1.0, 1.0085885098415446,
       1.0, 1.1375361376887123, 1.0, 1.1375361376887123,
       1.0, 1.0, 1.0, 1.0]
_NCB = [0, 1, 2, 3, 4, 5, 6, 7, 8, 9, 12, 13]   # 16-bone padded columns
_RUNS_E1 = [(0, 4, 2, 1), (4, 2, 8, 1), (6, 2, 12, 1), (8, 1, 1, 1),
            (9, 1, 4, 1), (10, 1, 11, 1), (11, 1, 8, 1)]
_RUNS_E2 = [(0, 2, 1, 1), (2, 2, 5, 1), (4, 4, 9, 1), (8, 2, 0, 0),
            (10, 2, 7, 0)]
_RUNS_D = [(0, 2, 2, 1, 1, 1), (2, 2, 4, 1, 5, 1), (4, 2, 8, 1, 9, 1),
           (6, 2, 12, 1, 11, 1), (8, 1, 1, 1, 0, 1), (9, 1, 4, 1, 0, 1),
           (10, 1, 11, 1, 7, 1), (11, 1, 8, 1, 7, 1)]
_VAR_WEIGHT = 1.0

# gather call plan: (half, queue, slot0, nslots); queue q owns 7 slots of
# one half. Calls are capped at 2 slots (1024 descs) by the SWDGE
# descriptor-ring carveout (fixed by the runtime; larger calls hang in
# await_space).
_CALL_ROUNDS = [
    [(0, 0, 0, 2), (0, 1, 7, 2), (1, 2, 0, 2), (1, 3, 7, 2)],
    [(0, 0, 2, 2), (0, 1, 9, 2), (1, 2, 2, 2), (1, 3, 9, 2)],
    [(0, 0, 4, 2), (0, 1, 11, 2), (1, 2, 4, 2), (1, 3, 11, 2)],
    [(0, 0, 6, 1), (0, 1, 13, 1), (1, 2, 6, 1), (1, 3, 13, 1)],
]

_F32 = mybir.dt.float32
_I32 = mybir.dt.int32
_I16 = mybir.dt.int16


def _ap(base_ap, dims, off=0):
    """Custom AP: keep base partition dim, override free dims; offset in elems."""
    return bass.AP(base_ap.tensor, base_ap.offset + off,
                   [list(base_ap.ap[0])] + [list(d) for d in dims])


def _dap(base_ap, dims, off=0):
    """Custom DRAM AP with ALL dims explicit (first dim included)."""
    return bass.AP(base_ap.tensor, base_ap.offset + off,
                   [list(d) for d in dims])


def _consts():
    u = np.arange(32, dtype=np.int32)
    p16 = np.arange(16, dtype=np.int32)
    # (16u + p) * 4096: (ind + s'*4096) >> 6 == (ind>>6) + s'*64
    c_base2 = ((16 * u[None, :] + p16[:, None]) * 4096).astype(np.int32)
    c_all = np.zeros((P, 81), np.float32)
    c_all[:, 0:64] = np.arange(64, dtype=np.float32)[None, :]
    w2 = np.zeros(16, np.float32)
    for b, c in enumerate(_NCB):
        w2[c] = np.float32(_WB[b]) ** 2
    c_all[:, 64:80] = w2[None, :]
    c_all[:, 80] = 1.0
    return {"c_base2": c_base2, "c_all": c_all}


def _build_nc():
    nc = bacc.Bacc("TRN2", target_bir_lowering=False, debug=False,
                   enable_asserts=False, num_devices=NCORES,
                   num_swdge_queues=4)
    outv = nc.dram_tensor("outv", [S * 64, 64], _F32, kind="ExternalInput").ap()
    packv = nc.dram_tensor("packv", [S, PK], _I32, kind="ExternalInput").ap()
    c_base2 = nc.dram_tensor("c_base2", [16, 32], _I32,
                             kind="ExternalInput").ap()
    c_all = nc.dram_tensor("c_all", [P, 81], _F32, kind="ExternalInput").ap()
    res = nc.dram_tensor("res", [1, 8], _F32, kind="ExternalOutput").ap()

    AL = mybir.AluOpType
    X = mybir.AxisListType.X
    with tile.TileContext(nc) as tc:
        with tc.tile_pool(name="sbuf", bufs=1) as pool, \
             tc.tile_pool(name="psum", bufs=1, space="PSUM") as psum_pool:
            from concourse import library_config
            nc.gpsimd.load_library(library_config.mlp)

            base2_t = pool.tile([16, 32], _I32)
            call_t = pool.tile([P, 81], _F32)
            iota_v = call_t[:, 0:64]
            zidx_t = pool.tile([32, 8], _I16)
            dump_t = pool.tile([P, 64], _F32)

            idx0 = pool.tile([128, 448], _I16, tag="idx0")
            idx1 = pool.tile([128, 448], _I16, tag="idx1")
            idx_tiles = {0: idx0, 1: idx1}
            g0 = pool.tile([P, 3584], _F32, tag="g0")
            g1 = pool.tile([P, 3584], _F32, tag="g1")
            g_tiles = {0: g0, 1: g1}
            eqm0 = pool.tile([P, 3584], _F32, tag="eqm0")
            eqm1 = pool.tile([P, 3584], _F32, tag="eqm1")
            eqm_tiles = {0: eqm0, 1: eqm1}
            b_pred = pool.tile([P, 112], _F32)
            lof = pool.tile([P, 112], _F32)
            b_all = pool.tile([P, 816], _F32)   # [h*408 + b*102 + c]
            t1raw = pool.tile([16, 2176], _I32)
            bg_tv = pool.tile([P, 768], _F32)
            pd = pool.tile([P, 128], _F32)

            # (no dummy gather: all 8 Q7 cores page the mlp library while
            # decoding the FIRST real gather — non-owning cores idle-respond
            # after the page-in, so the library load is inherently parallel)

            # ----- phase 0.1: input DMAs (one per queue-slot where possible)
            with tc.tile_wait_until(0.1):
                # half 0 of the sample-wrapped ind load is split across both
                # HWDGE queues: its sem gates the whole idx chain
                nc.sync.dma_start(
                    _ap(t1raw[:], [[34, 16], [1, 34]], off=0),
                    _dap(packv[:], [[PK, 16], [16 * PK, 16], [1, 34]], off=0))
                nc.scalar.dma_start(
                    _ap(t1raw[:], [[34, 16], [1, 34]], off=544),
                    _dap(packv[:], [[PK, 16], [16 * PK, 16], [1, 34]],
                         off=16 * 16 * PK))
                nc.scalar.dma_start(
                    _ap(t1raw[:], [[34, 32], [1, 34]], off=1088),
                    _dap(packv[:], [[PK, 16], [16 * PK, 32], [1, 34]],
                         off=512 * PK))
                nc.sync.dma_start(base2_t[:], c_base2[:])
                nc.sync.dma_start(call_t[:], c_all[:])
                # pack loads run while the scalar queue would otherwise idle
                # waiting on the idx casts that gate the h1 replicas
                for h in range(2):
                    nc.scalar.dma_start(
                        _ap(b_all[:], [[102, 4], [1, PK]], off=h * 408),
                        _dap(packv[:], [[PK, 128], [128 * PK, 4], [1, PK]],
                             off=(512 * h) * PK).bitcast(_F32))

            # ----- phase 0.15: dummy-bone zeroing ---------------------------
            with tc.tile_wait_until(0.15):
                nc.vector.memset(bg_tv[:], 0.0)
                nc.vector.memset(pd[:], 0.0)

            # ----- phase 0.2: idx math + casts (h0 first), replication ------
            with tc.tile_wait_until(0.2):
                for h in range(2):
                    ev = _ap(t1raw[:], [[34, 32], [2, 17]], off=h * 1088)
                    nc.vector.tensor_tensor(
                        out=ev, in0=ev,
                        in1=_ap(base2_t[:], [[1, 32], [0, 17]]), op=AL.add)
                    nc.vector.tensor_scalar(out=ev, in0=ev, scalar1=6,
                                            scalar2=None,
                                            op0=AL.logical_shift_right)
                    it = idx_tiles[h]
                    for ci, (jt, j0, cnt) in enumerate(_CHUNKS_ALL):
                        nc.vector.tensor_copy(
                            out=_ap(it[0:16, :], [[32, cnt], [1, 32]],
                                    off=jt * 32),
                            in_=_ap(t1raw[:], [[2, cnt], [34, 32]],
                                    off=h * 1088 + 2 * j0))
                        if h == 0 and ci == 0:
                            # partial [16:32] replica right after the first
                            # cast: it alone gates round-1 q0 (cols 0:64),
                            # pulling the whole gather pipeline earlier
                            nc.sync.dma_start(it[16:32, 0:224],
                                              it[0:16, 0:224])
                # replicate [0:16] to every other 16-partition group with
                # independent flat DMAs (no chained sem latency)
                nc.sync.dma_start(idx0[16:32, 224:448], idx0[0:16, 224:448])
                for p0 in range(32, 128, 16):
                    nc.sync.dma_start(idx0[p0:p0 + 16, 0:448],
                                      idx0[0:16, 0:448])
                for p0 in range(16, 128, 16):
                    nc.scalar.dma_start(idx1[p0:p0 + 16, 0:448],
                                        idx1[0:16, 0:448])

            # ----- phase 0.3: lo = ind & 63 ---------------------------------
            with tc.tile_wait_until(0.3):
                for h in range(2):
                    iv = _ap(b_all[:], [[102, 4], [2, 17]],
                             off=h * 408).bitcast(_I32)
                    nc.vector.tensor_scalar(out=iv, in0=iv, scalar1=63,
                                            scalar2=None, op0=AL.bitwise_and)
                    for (jt, j0, cnt) in _CHUNKS_ALL:
                        nc.vector.tensor_copy(
                            out=_ap(lof[:], [[4, cnt], [1, 4]],
                                    off=h * 56 + jt * 4),
                            in_=_ap(b_all[:], [[2, cnt], [102, 4]],
                                    off=h * 408 + 2 * j0).bitcast(_I32))

            # ----- phase 0.5: eq-mask precompute (hidden under gathers) -----
            with tc.tile_wait_until(0.5):
                for h in range(2):
                    nc.vector.tensor_tensor(
                        out=eqm_tiles[h][:].rearrange("p (a e) -> p a e", e=64),
                        in0=_ap(iota_v, [[0, 56], [1, 64]]),
                        in1=_ap(lof[:], [[1, 56], [0, 64]], off=h * 56),
                        op=AL.is_equal)

            # ----- phase 0.6: early bone math (target/gt_2d only) -----------
            xy2 = pool.tile([P, 128], _F32)
            vw2 = pool.tile([P, 128], _F32)
            rn = pool.tile([P, 32], _F32)
            gv = pool.tile([P, 128], _F32)
            with tc.tile_wait_until(0.6):
                for e, runs in enumerate((_RUNS_E1, _RUNS_E2)):
                    for (b0, ln, q0, st) in runs:
                        j0 = _JL[q0]
                        nc.vector.tensor_copy(
                            out=_ap(bg_tv[:], [[8, ln], [4, 2], [1, 4]],
                                    off=e * 128 + _NCB[b0] * 8),
                            in_=_ap(b_all[:], [[st, ln], [408, 2], [102, 4]],
                                    off=34 + j0))
                        nc.vector.tensor_copy(
                            out=_ap(bg_tv[:], [[16, ln], [8, 2], [4, 2], [1, 4]],
                                    off=256 + e * 256 + _NCB[b0] * 16),
                            in_=_ap(b_all[:], [[2 * st, ln], [1, 2], [408, 2],
                                               [102, 4]],
                                    off=51 + 2 * j0))
                n128 = [[1, 128]]
                vis = pool.tile([P, 128], _F32)
                v2 = pool.tile([P, 128], _F32)
                nc.vector.tensor_scalar(out=vis[:], in0=_ap(bg_tv[:], n128, off=0),
                                        scalar1=0.5, scalar2=None, op0=AL.is_gt)
                nc.vector.tensor_scalar(out=v2[:],
                                        in0=_ap(bg_tv[:], n128, off=128),
                                        scalar1=0.5, scalar2=None, op0=AL.is_gt)
                nc.vector.tensor_tensor(out=vis[:], in0=vis[:], in1=v2[:],
                                        op=AL.mult)
                # vw2 = vis * w^2  (fold bone weight into d2 before the sqrt)
                nc.vector.tensor_tensor(
                    out=vw2[:].rearrange("p (a b) -> p a b", a=16),
                    in0=vis[:].rearrange("p (a b) -> p a b", a=16),
                    in1=_ap(call_t[:], [[1, 16], [0, 8]], off=64), op=AL.mult)
                dx = pool.tile([P, 128], _F32)
                dy = pool.tile([P, 128], _F32)
                nc.vector.tensor_tensor(
                    out=dx[:].rearrange("p (a b) -> p a b", a=16),
                    in0=_ap(bg_tv[:], [[16, 16], [1, 8]], off=256),
                    in1=_ap(bg_tv[:], [[16, 16], [1, 8]], off=512),
                    op=AL.subtract)
                nc.vector.tensor_tensor(
                    out=dy[:].rearrange("p (a b) -> p a b", a=16),
                    in0=_ap(bg_tv[:], [[16, 16], [1, 8]], off=264),
                    in1=_ap(bg_tv[:], [[16, 16], [1, 8]], off=520),
                    op=AL.subtract)
                nc.vector.tensor_tensor(out=dx[:], in0=dx[:], in1=dx[:],
                                        op=AL.mult)
                nc.vector.tensor_tensor(out=dy[:], in0=dy[:], in1=dy[:],
                                        op=AL.mult)
                nc.vector.tensor_tensor(out=xy2[:], in0=dx[:], in1=dy[:],
                                        op=AL.add)
                # num = bones visible per group; rn = 1/max(num, 1)
                num = pool.tile([P, 32], _F32)
                nc.vector.tensor_reduce(
                    out=_ap(num[:], [[8, 4], [1, 8]]),
                    in_=_ap(vis[:], [[32, 4], [1, 8], [8, 4]]),
                    axis=X, op=AL.add)
                nc.vector.tensor_scalar(out=num[:], in0=num[:], scalar1=1.0,
                                        scalar2=None, op0=AL.max)
                nc.vector.reciprocal(out=rn[:], in_=num[:])
                # active-sample mask from packed mask columns
                msum = pool.tile([P, 8], _F32)
                nc.vector.tensor_reduce(
                    out=msum[:],
                    in_=_ap(b_all[:], [[102, 8], [1, 17]], off=85),
                    axis=X, op=AL.add)
                nc.vector.tensor_scalar(out=msum[:], in0=msum[:], scalar1=0.0,
                                        scalar2=None, op0=AL.is_equal)
                rn_m = pool.tile([P, 32], _F32)
                nc.vector.tensor_tensor(out=rn_m[:], in0=rn[:],
                                        in1=_ap(msum[:], [[0, 4], [1, 8]]),
                                        op=AL.mult)
                nc.vector.tensor_tensor(
                    out=_ap(gv[:], [[32, 4], [8, 4], [1, 8]]),
                    in0=_ap(vis[:], [[32, 4], [8, 4], [1, 8]]),
                    in1=_ap(rn_m[:], [[8, 4], [0, 4], [1, 8]]), op=AL.mult)

            # ----- gathers: 3 rounds x 4 queues -----------------------------
            def emit_gather(h, q, s0, ns, ph):
                with tc.tile_wait_until(ph):
                    nc.gpsimd.dma_gather(
                        _ap(g_tiles[h][:], [[64, ns * 4], [1, 64]],
                            off=s0 * 256),
                        outv[h * HS * 64:(h + 1) * HS * 64, :],
                        idx_tiles[h][0:32 * (q + 1), s0 * 32:(s0 + ns) * 32],
                        ns * 512, ns * 512, 64, elem_step=64,
                        queue_num=q,
                    )

            def emit_select(h, s0, ns, ph):
                # one mult + one reduce cover BOTH queues of a half: queue
                # q+1's slots sit exactly 7 slots (1792 cols) after queue q's
                with tc.tile_wait_until(ph):
                    eview = _ap(eqm_tiles[h][:], [[1792, 2], [1, ns * 256]],
                                off=s0 * 256)
                    nc.vector.tensor_tensor(
                        out=eview, in0=eview,
                        in1=_ap(g_tiles[h][:], [[1792, 2], [1, ns * 256]],
                                off=s0 * 256),
                        op=AL.mult)
                    nc.vector.tensor_reduce(
                        out=_ap(b_pred[:], [[28, 2], [1, ns * 4]],
                                off=h * 56 + s0 * 4),
                        in_=_ap(eqm_tiles[h][:], [[1792, 2], [64, ns * 4],
                                                  [1, 64]],
                                off=s0 * 256),
                        axis=X, op=AL.add)

            for ri, calls in enumerate(_CALL_ROUNDS):
                for ci, (h, q, s0, ns) in enumerate(calls):
                    emit_gather(h, q, s0, ns, 1.0 + ri + 0.01 * ci)
            for ri, calls in enumerate(_CALL_ROUNDS):
                for hi, (h, q, s0, ns) in enumerate(calls[::2]):
                    emit_select(h, s0, ns, 1.5 + ri + 0.01 * hi)

            # ----- late bone math (needs pred) ------------------------------
            with tc.tile_wait_until(6.0):
                for (b0, ln, p1, s1, p2, s2) in _RUNS_D:
                    nc.vector.tensor_tensor(
                        out=_ap(pd[:], [[8, ln], [4, 2], [1, 4]],
                                off=_NCB[b0] * 8),
                        in0=_ap(b_pred[:], [[4 * s1, ln], [56, 2], [1, 4]],
                                off=p1 * 4),
                        in1=_ap(b_pred[:], [[4 * s2, ln], [56, 2], [1, 4]],
                                off=p2 * 4),
                        op=AL.subtract)
                d2 = pool.tile([P, 128], _F32)
                nc.vector.tensor_tensor(out=d2[:], in0=pd[:], in1=pd[:],
                                        op=AL.mult)
                nc.vector.tensor_tensor(out=d2[:], in0=d2[:], in1=xy2[:],
                                        op=AL.add)
                nc.vector.tensor_tensor(out=d2[:], in0=d2[:], in1=vw2[:],
                                        op=AL.mult)
                ell = pool.tile([P, 128], _F32)
                nc.scalar.sqrt(out=ell[:], in_=d2[:])
                # gate = (d2 > 0) * gv, on DVE while ACT does the sqrt
                gt = pool.tile([P, 128], _F32)
                nc.vector.tensor_scalar(out=gt[:], in0=d2[:], scalar1=0.0,
                                        scalar2=None, op0=AL.is_gt)
                nc.vector.tensor_tensor(out=gt[:], in0=gt[:], in1=gv[:],
                                        op=AL.mult)
                sum_l = pool.tile([P, 32], _F32)
                nc.vector.tensor_reduce(
                    out=_ap(sum_l[:], [[8, 4], [1, 8]]),
                    in_=_ap(ell[:], [[32, 4], [1, 8], [8, 4]]),
                    axis=X, op=AL.add)
                e_t = pool.tile([P, 32], _F32)
                nc.vector.tensor_tensor(out=e_t[:], in0=sum_l[:], in1=rn[:],
                                        op=AL.mult)
                eb = pool.tile([P, 128], _F32)
                nc.vector.tensor_tensor(
                    out=_ap(eb[:], [[32, 4], [8, 4], [1, 8]]),
                    in0=_ap(ell[:], [[32, 4], [8, 4], [1, 8]]),
                    in1=_ap(e_t[:], [[8, 4], [0, 4], [1, 8]]), op=AL.subtract)
                nc.vector.tensor_tensor(out=eb[:], in0=eb[:], in1=eb[:],
                                        op=AL.mult)
                nc.vector.tensor_tensor(out=eb[:], in0=eb[:], in1=gt[:],
                                        op=AL.mult)
                pl = pool.tile([P, 8], _F32)
                nc.vector.tensor_reduce(out=pl[:],
                                        in_=_ap(eb[:], [[1, 8], [8, 16]]),
                                        axis=X, op=AL.add)
                ps = psum_pool.tile([1, 8], _F32, space="PSUM")
                nc.tensor.matmul(out=ps[:], lhsT=call_t[:, 80:81], rhs=pl[:],
                                 start=True, stop=True)
                tot = pool.tile([1, 8], _F32)
                nc.vector.tensor_copy(out=tot[:], in_=ps[:])
                nc.sync.dma_start(res[:], tot[0:1, :])
    nc.compile()
    return nc


_NC_CACHE = None
LAST_RESULTS = None


def kernel(output, mask, ind, target, gt_2d):
    global _NC_CACHE, LAST_RESULTS
    if _NC_CACHE is None:
        _NC_CACHE = _build_nc()
    nc = _NC_CACHE

    output = np.ascontiguousarray(np.asarray(output), dtype=np.float32)
    mask = np.ascontiguousarray(np.asarray(mask), dtype=np.float32)
    target = np.ascontiguousarray(np.asarray(target), dtype=np.float32)
    gt_2d = np.ascontiguousarray(
        np.asarray(gt_2d), dtype=np.float32).reshape(B, 34)
    ind = np.ascontiguousarray(np.asarray(ind))
    if ind.dtype != np.int64:
        ind = ind.astype(np.int64)

    pack = np.empty((B, PK), np.int32)
    pack[:, 0:34] = ind.view(np.int32).reshape(B, 34)
    pack[:, 34:51] = target.view(np.int32)
    pack[:, 51:85] = np.ascontiguousarray(gt_2d).view(np.int32)
    pack[:, 85:102] = mask.view(np.int32)

    consts = _consts()
    in_maps = []
    for c in range(NCORES):
        sl = slice(c * S, (c + 1) * S)
        in_maps.append({
            "outv": np.ascontiguousarray(output[sl]).reshape(S * 64, 64),
            "packv": np.ascontiguousarray(pack[sl]),
            **consts,
        })
    res = run_bass_kernel_spmd(nc, in_maps, core_ids=list(range(NCORES)))
    LAST_RESULTS = res
    total = sum(float(res.results[c]["res"].sum()) for c in range(NCORES))
    return np.asarray([_VAR_WEIGHT * total * 0.5 / B], dtype=np.float32)
